# revision 41
# baseline (speedup 1.0000x reference)
"""Trainium2 Bass kernel for GemNet AtomUpdateBlock (gnn_message_passing).

Computation (per reference):
    bases = basis_rad @ W_rbf              # [E, De]
    x     = m * bases                      # [E, De]
    z     = segment_sum(x, idx_atom, A)    # [A, De]
    x     = silu(z @ W_in)                 # [A, Da]
    3x residual: x = (x + silu(silu(x W1) W2)) / sqrt(2)

Distribution strategy: shard EDGES BY DESTINATION ATOM. The host bins the
atoms into 8 cores x T_ATOM tiles of <=128 atoms (balanced by edge count),
sorts/pads each tile's edges into K 128-edge groups, and each core computes
the segment-sum + atom MLP for its own atoms only. No collective needed;
outputs are disjoint atom slices.

v2 (vs v1 baseline at ~292-347us):
 - bases matmuls (K=16) are 2-way ROW-TILED: two consecutive 128-edge
   groups' basis blocks sit at partitions 0-15 / 32-47 and run as
   concurrent 32-row PE tiles into two PSUM banks of one [P,1024] tile.
 - the x = m*bases multiply is split across engines per pack:
   route A: ACT copies the [128,1024] f32 PSUM pair to SBUF bf16, then
   DVE (or GPSIMD for one pack per subtile) does a bf16 2x-rate TT.
   route B: DVE does the TT directly from PSUM at 1x rate.
   A 3-stage software pipeline (bases -> evac/mult -> scatter) keeps the
   PE dense so the HAM clock gate stays at 8/8.
 - z evac: PSUM->SBUF bf16 copy, 4 bf16 PE transposes into one PSUM bank,
   one dense DVE copy out (zt layout is subtile-major; the MLP matmuls
   read it with a strided rhs AP).
 - epilogue u/v/w PSUM tiles are [P,1024] so each silu is one batched
   [128,1024] ACT instruction; residual skip-adds and the final 1/sqrt(2)^3
   scale run on the otherwise-idle GPSIMD engine.
"""

import math
import os
import sys

import numpy as np
import ml_dtypes

BF16 = ml_dtypes.bfloat16

P = 128
N_CORES = 8
DE, DA, DR, NH = 512, 256, 16, 3
T_ATOM = 20  # atom tiles per core (each up to 128 atoms); divisible by 4
INV_SQRT_2 = 0.7071067811865476

_NC_CACHE = {}
SILU_NATIVE = True  # False: sigmoid+mult (for CoreSim, which lacks Silu)


# ----------------------------------------------------------------------------
# Host-side packing
# ----------------------------------------------------------------------------

def _pack_layout(idx, n_atoms, n_cores, t_atom):
    E = idx.shape[0]
    n_bins = n_cores * t_atom
    counts = np.bincount(idx, minlength=n_atoms)

    order = np.argsort(-counts, kind="stable")
    n_rounds = math.ceil(n_atoms / n_bins)
    pad = n_rounds * n_bins - n_atoms
    padded = np.concatenate([order, np.full(pad, -1, dtype=order.dtype)])
    grid = padded.reshape(n_rounds, n_bins)
    grid[1::2] = grid[1::2, ::-1]  # snake-deal: balances edges and atoms
    bin_of_atom = np.empty(n_atoms, dtype=np.int64)
    slot_of_atom = np.empty(n_atoms, dtype=np.int64)
    valid = grid >= 0
    bin_idx = np.broadcast_to(np.arange(n_bins), grid.shape)
    round_idx = np.broadcast_to(np.arange(n_rounds)[:, None], grid.shape)
    bin_of_atom[grid[valid]] = bin_idx[valid]
    slot_of_atom[grid[valid]] = round_idx[valid]
    assert np.bincount(bin_of_atom, minlength=n_bins).max() <= P

    ebin = bin_of_atom[idx]
    eslot = slot_of_atom[idx]
    eorder = np.argsort(ebin * (P + 1) + eslot, kind="stable")
    ebin_sorted = ebin[eorder]
    bin_counts = np.bincount(ebin_sorted, minlength=n_bins)
    K = max(1, math.ceil(bin_counts.max() / P))
    bin_starts = np.zeros(n_bins + 1, dtype=np.int64)
    np.cumsum(bin_counts, out=bin_starts[1:])
    pos_in_bin = np.arange(E) - bin_starts[ebin_sorted]

    core_of_bin = np.arange(n_bins) // t_atom
    tile_of_bin = np.arange(n_bins) % t_atom
    return dict(
        K=K,
        eorder=eorder,
        core_of_edge=core_of_bin[ebin_sorted],
        flat_slot=tile_of_bin[ebin_sorted] * (K * P) + pos_in_bin,
        rel_of_edge=eslot[eorder].astype(np.int64),
        bin_of_atom=bin_of_atom,
        slot_of_atom=slot_of_atom,
        core_of_bin=core_of_bin,
        tile_of_bin=tile_of_bin,
    )


def _pack_weights(W_rbf, W_in, res_W1, res_W2):
    Ci, Cj = DE // P, DA // P
    Cr = DA // P
    win = W_in.reshape(Ci, P, Cj, P).transpose(1, 0, 2, 3).reshape(P, Ci * Cj * P)
    blocks = []
    c = INV_SQRT_2
    for l in range(NH):
        w1 = (res_W1[l] * (c ** l)).astype(np.float32)
        w2 = res_W2[l].astype(np.float32)
        for W in (w1, w2):
            blocks.append(
                W.reshape(Cr, P, Cr, P).transpose(1, 0, 2, 3).reshape(P, Cr * Cr * P)
            )
    wres = np.concatenate(blocks, axis=1)
    # W_rbf replicated at partitions 0-15 and 32-47 for 2-way row tiling
    wrbf2 = np.zeros((64, DE), dtype=BF16)
    wrbf2[0:DR] = W_rbf.astype(BF16)
    wrbf2[32:32 + DR] = W_rbf.astype(BF16)
    return (
        np.ascontiguousarray(wrbf2),
        np.ascontiguousarray(win, dtype=BF16),
        np.ascontiguousarray(wres, dtype=BF16),
    )


def _build_in_maps(m, basis_rad, layout, W_rbf, W_in, res_W1, res_W2, n_cores, t_atom):
    K = layout["K"]
    cap = t_atom * K * P
    ncols = t_atom * K
    npairs = K // 2
    npacks = npairs + (K % 2)
    eorder = layout["eorder"]
    core_of_edge = layout["core_of_edge"]
    flat_slot = layout["flat_slot"]
    rel = layout["rel_of_edge"]

    wrbf2, win, wres = _pack_weights(W_rbf, W_in, res_W1, res_W2)
    m_src = m[eorder]
    bas_src = basis_rad[eorder]
    ident = np.eye(P, dtype=BF16)

    in_maps = []
    for c in range(n_cores):
        sel = core_of_edge == c
        fs = flat_slot[sel]
        m_pack = np.zeros((cap, DE), dtype=BF16)
        m_pack[fs] = m_src[sel].astype(BF16)
        # partition-major: m2[p, col*DE + d] = m_pack[col*P + p, d] so each
        # partition's per-atom-tile DMA read is fully contiguous
        m_pack = np.ascontiguousarray(
            m_pack.reshape(ncols, P, DE).transpose(1, 0, 2).reshape(P, ncols * DE)
        )
        basT = np.zeros((DR, cap), dtype=BF16)
        basT[:, fs] = bas_src[sel].T.astype(BF16)
        # 2-way row-tiled layout: pack j of tile t holds group 2j at
        # partitions 0-15 and group 2j+1 (if any) at partitions 32-47.
        bas_pack = np.zeros((64, t_atom * npacks * P), dtype=BF16)
        for t in range(t_atom):
            for j in range(npacks):
                col0 = (t * npacks + j) * P
                k = 2 * j
                bas_pack[0:DR, col0:col0 + P] = basT[
                    :, (t * K + k) * P:(t * K + k + 1) * P
                ]
                if k + 1 < K:
                    bas_pack[32:32 + DR, col0:col0 + P] = basT[
                        :, (t * K + k + 1) * P:(t * K + k + 2) * P
                    ]
        rel_flat = np.full(cap, -1, dtype=np.int64)
        rel_flat[fs] = rel[sel]
        rel2 = rel_flat.reshape(ncols, P).T  # [p, col]
        s_host = (rel2[:, :, None] == np.arange(P)[None, None, :]).astype(BF16)
        in_maps.append(
            dict(
                m_pack=m_pack,
                bas_pack=np.ascontiguousarray(bas_pack),
                s_hot=np.ascontiguousarray(s_host.reshape(P, ncols * P)),
                wrbf2=wrbf2,
                win=win,
                wres=wres,
                ident=ident,
            )
        )
    return in_maps


def _unpack_output(results, layout, n_atoms, n_cores, t_atom):
    Cj = DA // P
    out = np.zeros((n_atoms, DA), dtype=np.float32)
    core_of_atom = layout["core_of_bin"][layout["bin_of_atom"]]
    row_of_atom = (
        layout["tile_of_bin"][layout["bin_of_atom"]] * P + layout["slot_of_atom"]
    )
    for c in range(n_cores):
        x = results[c]["out"].reshape(P, Cj, t_atom, P)
        x_core = x.transpose(2, 3, 1, 0).reshape(t_atom * P, DA)
        mask = core_of_atom == c
        out[mask] = x_core[row_of_atom[mask]]
    return out


# ----------------------------------------------------------------------------
# Bass kernel builder
# ----------------------------------------------------------------------------

def _build_nc(t_atom, K):
    import concourse.mybir as mybir
    import concourse.tile as tile
    from concourse import bacc

    f32 = mybir.dt.float32
    bf16 = mybir.dt.bfloat16
    Ci, Cj = DE // P, DA // P
    Cr = DA // P
    npairs = K // 2
    npacks = npairs + (K % 2)
    C3 = INV_SQRT_2 ** NH
    GAMMA = [float((1.0 / INV_SQRT_2) ** l) for l in range(NH)]
    assert t_atom % 4 == 0
    n_quads = t_atom // 4
    W4 = 4 * P  # atoms per epilogue quad

    nc = bacc.Bacc(
        "TRN2",
        target_bir_lowering=False,
        debug=False,
        enable_asserts=False,
        num_devices=N_CORES,
    )
    d_m = nc.dram_tensor("m_pack", [P, t_atom * K * DE], bf16, kind="ExternalInput")
    d_bas = nc.dram_tensor(
        "bas_pack", [64, t_atom * npacks * P], bf16, kind="ExternalInput"
    )
    d_s = nc.dram_tensor("s_hot", [P, t_atom * K * P], bf16, kind="ExternalInput")
    d_wrbf2 = nc.dram_tensor("wrbf2", [64, DE], bf16, kind="ExternalInput")
    d_win = nc.dram_tensor("win", [P, Ci * Cj * P], bf16, kind="ExternalInput")
    d_wres = nc.dram_tensor(
        "wres", [P, NH * 2 * Cr * Cr * P], bf16, kind="ExternalInput"
    )
    d_ident = nc.dram_tensor("ident", [P, P], bf16, kind="ExternalInput")
    d_out = nc.dram_tensor("out", [P, Cj * t_atom * P], f32, kind="ExternalOutput")

    with tile.TileContext(nc) as tc:
        with (
            tc.tile_pool(name="const", bufs=1) as const_p,
            tc.tile_pool(name="bas", bufs=5) as bas_p,
            tc.tile_pool(name="m", bufs=5) as m_p,
            tc.tile_pool(name="x", bufs=4) as x_p,
            tc.tile_pool(name="bsb", bufs=3) as bsb_p,
            tc.tile_pool(name="s", bufs=5) as s_p,
            tc.tile_pool(name="zsb", bufs=3) as zsb_p,
            tc.tile_pool(name="ztsb", bufs=2) as ztsb_p,
            tc.tile_pool(name="act", bufs=4) as act_p,
            tc.tile_pool(name="outp", bufs=3) as out_p,
            tc.tile_pool(name="ps_z", bufs=2, space="PSUM") as psz_p,
            tc.tile_pool(name="ps_bases", bufs=2, space="PSUM") as psb_p,
            tc.tile_pool(name="ps_misc", bufs=1, space="PSUM") as psm_p,
        ):
            # Resident constants
            wrbf_sb = const_p.tile([64, DE], bf16, tag="wrbf2")
            nc.sync.dma_start(out=wrbf_sb[:], in_=d_wrbf2[:])
            win_sb = const_p.tile([P, Ci * Cj * P], bf16, tag="win")
            nc.sync.dma_start(out=win_sb[:], in_=d_win[:])
            wres_sb = const_p.tile([P, NH * 2 * Cr * Cr * P], bf16, tag="wres")
            nc.sync.dma_start(out=wres_sb[:], in_=d_wres[:])
            ident = const_p.tile([P, P], bf16, tag="ident")
            nc.sync.dma_start(out=ident[:], in_=d_ident[:])

            # HAM warmup: dense back-to-back matmuls on resident weights
            # upclock the PE (4/8 -> 8/8) while the first DMAs stream.
            warm_ps = psm_p.tile(
                [P, Cj * W4], f32, space="PSUM", tag="misc", name="warm"
            )
            for w in range(20):
                nc.tensor.matmul(
                    out=warm_ps[:, 0:W4],
                    lhsT=win_sb[:, (w % 8) * P : (w % 8 + 1) * P],
                    rhs=win_sb[:, 0:W4],
                    start=True,
                    stop=True,
                )

            _ctr = [0]

            def emit_silu(out_ap, in_ps_ap):
                if SILU_NATIVE:
                    nc.scalar.activation(
                        out=out_ap, in_=in_ps_ap,
                        func=mybir.ActivationFunctionType.Silu,
                    )
                else:
                    _ctr[0] += 1
                    sg = act_p.tile(
                        [P, in_ps_ap.free_size()], f32, tag="sig",
                        name=f"sig{_ctr[0]}"
                    )
                    nc.scalar.activation(
                        out=sg[:], in_=in_ps_ap,
                        func=mybir.ActivationFunctionType.Sigmoid,
                    )
                    nc.vector.tensor_tensor(
                        out=out_ap, in0=in_ps_ap, in1=sg[:],
                        op=mybir.AluOpType.mult,
                    )

            def epilogue_gen(q, zt_sb, s0=0, ns=4, delay=5):
                """Epilogue over atom columns of subtiles [s0, s0+ns) of quad
                q, emitted as units interleavable with the edge stream. The
                epilogue is column-separable by subtile, so the last quad
                runs it as per-subtile chains that start right after each
                subtile's evac instead of after the whole quad."""
                ztr = zt_sb.rearrange("p (s c x) -> p s c x", s=4, c=Ci)
                W = ns * P  # atom columns processed by this chain
                # Delay the first u matmul a few packs so the source zt
                # (previous quad's last-subtile evac) has completed and the
                # in-order PE queue never blocks on it.
                for _ in range(delay):
                    yield
                u_ps = psm_p.tile(
                    [P, Cj * W], f32, space="PSUM", tag="misc",
                    name=f"ups{q}_{s0}"
                )
                for j in range(Cj):
                    for c in range(Ci):
                        fi = c * Cj + j
                        nc.tensor.matmul(
                            out=u_ps[:, j * W:(j + 1) * W],
                            lhsT=win_sb[:, fi * P : (fi + 1) * P],
                            rhs=ztr[:, s0 : s0 + ns, c, :],
                            start=(c == 0),
                            stop=(c == Ci - 1),
                        )
                        yield
                X = act_p.tile([P, Cj * W], bf16, tag="X", name=f"X{q}_{s0}_0")
                emit_silu(X[:], u_ps[:])
                yield
                yield
                yield
                for l in range(NH):
                    v_ps = psm_p.tile(
                        [P, Cr * W], f32, space="PSUM", tag="misc",
                        name=f"vps{q}_{s0}_{l}"
                    )
                    for j in range(Cr):
                        for i in range(Cr):
                            fi = ((l * 2 + 0) * Cr + i) * Cr + j
                            nc.tensor.matmul(
                                out=v_ps[:, j * W:(j + 1) * W],
                                lhsT=wres_sb[:, fi * P : (fi + 1) * P],
                                rhs=X[:, i * W : (i + 1) * W],
                                start=(i == 0),
                                stop=(i == Cr - 1),
                            )
                            yield
                    u1 = act_p.tile(
                        [P, Cr * W], bf16, tag="X", name=f"u1_{q}_{s0}_{l}"
                    )
                    emit_silu(u1[:], v_ps[:])
                    yield
                    yield
                    w_ps = psm_p.tile(
                        [P, Cr * W], f32, space="PSUM", tag="misc",
                        name=f"wps{q}_{s0}_{l}"
                    )
                    for j in range(Cr):
                        for i in range(Cr):
                            fi = ((l * 2 + 1) * Cr + i) * Cr + j
                            nc.tensor.matmul(
                                out=w_ps[:, j * W:(j + 1) * W],
                                lhsT=wres_sb[:, fi * P : (fi + 1) * P],
                                rhs=u1[:, i * W : (i + 1) * W],
                                start=(i == 0),
                                stop=(i == Cr - 1),
                            )
                            yield
                    Y = act_p.tile(
                        [P, Cr * W], bf16, tag="X", name=f"Y{q}_{s0}_{l}"
                    )
                    emit_silu(Y[:], w_ps[:])
                    yield
                    yield
                    Xn = act_p.tile(
                        [P, Cr * W], bf16, tag="X", name=f"X{q}_{s0}_{l + 1}"
                    )
                    nc.vector.scalar_tensor_tensor(
                        out=Xn[:],
                        in0=Y[:],
                        scalar=GAMMA[l],
                        in1=X[:],
                        op0=mybir.AluOpType.mult,
                        op1=mybir.AluOpType.add,
                    )
                    X = Xn
                    yield
                o_t = out_p.tile([P, Cj * W], f32, tag="out", name=f"ot{q}_{s0}")
                nc.scalar.mul(out=o_t[:], in_=X[:], mul=float(C3))
                for j in range(Cj):
                    nc.sync.dma_start(
                        out=d_out[
                            :,
                            (j * t_atom + 4 * q + s0) * P
                            : (j * t_atom + 4 * q + s0 + ns) * P,
                        ],
                        in_=o_t[:, j * W : (j + 1) * W],
                    )
                yield

            # Pending epilogue generators, stepped strictly SEQUENTIALLY
            # (one exhausts before the next starts): concurrent generators
            # sharing the single-buffer psm pool deadlock the in-order PE
            # queue (gen B's tile alloc waits on gen A's not-yet-emitted
            # reader).
            epi_q = []

            def step_epis(n=1):
                for _ in range(n):
                    while epi_q:
                        if next(epi_q[0], StopIteration) is StopIteration:
                            epi_q.pop(0)
                            continue
                        break

            # Flat subtile stream: DMAs for subtile ts are emitted two
            # subtiles ahead of use so the sync queue never bunches a whole
            # quad's transfers at a quad boundary (that bunching caused
            # ~12us PE stalls at every quad edge).
            sub_state = {}

            def emit_subtile_dmas(ts):
                if ts >= 4 * n_quads or ts in sub_state:
                    return
                bas_sb = bas_p.tile(
                    [64, npacks * P], bf16, tag="bas", name=f"bas{ts}"
                )
                nc.sync.dma_start(
                    out=bas_sb[:],
                    in_=d_bas[:, ts * npacks * P : (ts + 1) * npacks * P],
                )
                m_t = m_p.tile([P, K * DE], bf16, tag="m", name=f"mt{ts}")
                nc.sync.dma_start(
                    out=m_t[:], in_=d_m[:, ts * K * DE : (ts + 1) * K * DE]
                )
                s_t = s_p.tile([P, K * P], bf16, tag="s", name=f"st{ts}")
                nc.sync.dma_start(
                    out=s_t[:], in_=d_s[:, ts * K * P : (ts + 1) * K * P]
                )
                z_ps = psz_p.tile(
                    [P, DE], f32, space="PSUM", tag="z", name=f"zps{ts}"
                )
                sub_state[ts] = (bas_sb, m_t, s_t, z_ps)

            for ts in range(3):
                emit_subtile_dmas(ts)

            for q in range(n_quads):
                subs = {}

                def do_evac(sub):
                    """z psum -> sbuf bf16 -> 4 bf16 PE transposes -> one
                    dense DVE copy into the quad's zt_sb (subtile-major)."""
                    t = 4 * q + sub
                    z_sb = zsb_p.tile([P, DE], bf16, tag="zsb", name=f"zsb{t}")
                    nc.scalar.copy(out=z_sb[:], in_=subs[sub][3][:])
                    zt_ps = psm_p.tile(
                        [P, DE], bf16, space="PSUM", tag="misc", name=f"ztp{t}"
                    )
                    for c in range(Ci):
                        nc.tensor.transpose(
                            out=zt_ps[:, c * P : (c + 1) * P],
                            in_=z_sb[:, c * P : (c + 1) * P],
                            identity=ident[:],
                        )
                    nc.vector.tensor_copy(
                        out=zt_sb[:, sub * DE : (sub + 1) * DE], in_=zt_ps[:]
                    )

                zt_sb = ztsb_p.tile([P, 4 * Ci * P], bf16, tag="ztsb")

                # pack list with routes:
                #  'A'  : ACT copies PSUM pair -> SBUF bf16, DVE does 2x TT
                #  'Agp': same but GPSIMD does the TT (one pack per subtile)
                #  'B'  : DVE tensor_tensor straight from PSUM (1x)
                packs = []
                for sub in range(4):
                    for j in range(npacks):
                        if j >= npairs:
                            route = "B"  # odd-K single group
                        elif j in (0, 2):
                            route = "Agp"
                        elif j == 4:
                            route = "A"
                        else:
                            route = "B"
                        packs.append((sub, j, route))

                def emit_bases(item, pb=None):
                    sub, j, route = item
                    bas_sb = subs[sub][0]
                    if pb is None:
                        pb = psb_p.tile(
                            [P, 2 * DE], f32, space="PSUM", tag="bases",
                            name=f"pb{q}_{sub}_{j}"
                        )
                    nc.tensor.matmul(
                        out=pb[:, 0:DE],
                        lhsT=bas_sb[0:DR, j * P : (j + 1) * P],
                        rhs=wrbf_sb[0:DR, :],
                        start=True,
                        stop=True,
                    )
                    if j < npairs:
                        nc.tensor.matmul(
                            out=pb[:, DE : 2 * DE],
                            lhsT=bas_sb[32 : 32 + DR, j * P : (j + 1) * P],
                            rhs=wrbf_sb[32 : 32 + DR, :],
                            start=True,
                            stop=True,
                        )
                    return pb

                def emit_mult(st):
                    """Stage 1: route A -> ACT copy; route B -> DVE TT."""
                    (sub, j, route), pb = st
                    ngr = 2 if j < npairs else 1
                    w = ngr * DE
                    m_t = subs[sub][1]
                    k0 = 2 * j
                    if route == "B":
                        x_t = x_p.tile(
                            [P, 2 * DE], bf16, tag="x", name=f"x{q}_{sub}_{j}"
                        )
                        nc.vector.tensor_tensor(
                            out=x_t[:, 0:w],
                            in0=pb[:, 0:w],
                            in1=m_t[:, k0 * DE : k0 * DE + w],
                            op=mybir.AluOpType.mult,
                        )
                        return x_t, None
                    bsb = bsb_p.tile(
                        [P, 2 * DE], bf16, tag="bsb", name=f"bsb{q}_{sub}_{j}"
                    )
                    nc.scalar.copy(out=bsb[:, 0:w], in_=pb[:, 0:w])
                    return None, bsb

                def emit_mult2(st2):
                    """Stage 2: route A's bf16 TT on DVE or GPSIMD."""
                    (sub, j, route), x_t, bsb = st2
                    if route == "B":
                        return x_t
                    ngr = 2 if j < npairs else 1
                    w = ngr * DE
                    m_t = subs[sub][1]
                    k0 = 2 * j
                    x_t = x_p.tile(
                        [P, 2 * DE], bf16, tag="x", name=f"x{q}_{sub}_{j}"
                    )
                    eng = nc.gpsimd if route == "Agp" else nc.vector
                    eng.tensor_tensor(
                        out=x_t[:, 0:w],
                        in0=bsb[:, 0:w],
                        in1=m_t[:, k0 * DE : k0 * DE + w],
                        op=mybir.AluOpType.mult,
                    )
                    return x_t

                last_q = q == n_quads - 1

                def emit_scatter(st3):
                    (sub, j, route), x_t = st3
                    s_t, z_ps = subs[sub][2], subs[sub][3]
                    ngr = 2 if j < npairs else 1
                    k0 = 2 * j
                    for g in range(ngr):
                        k = k0 + g
                        nc.tensor.matmul(
                            out=z_ps[:],
                            lhsT=s_t[:, k * P : (k + 1) * P],
                            rhs=x_t[:, g * DE : (g + 1) * DE],
                            start=(k == 0),
                            stop=(k == K - 1),
                        )
                    if j == npacks - 1:
                        do_evac(sub)
                        if last_q:
                            # last quad: queue this subtile's epilogue
                            # chain (column-separable) so it can start as
                            # soon as the previous epilogue exhausts, and
                            # the post-stream tail is ~one subtile deep.
                            epi_q.append(
                                epilogue_gen(q, zt_sb, s0=sub, ns=1, delay=1)
                            )

                q1, q2, q3 = [], [], []
                for pidx, item in enumerate(packs):
                    if item[1] == 0:
                        # entering a new subtile: its tiles were prefetched;
                        # kick off the DMAs for the subtile 3 ahead.
                        subs[item[0]] = sub_state[4 * q + item[0]]
                        emit_subtile_dmas(4 * q + item[0] + 3)
                    pb = emit_bases(item)
                    if q == 0 and 1 <= pidx <= 12:
                        # HAM warm-boost: duplicate the bases matmuls
                        # (idempotent PSUM overwrites) to keep the PE's
                        # activity window saturated during the DMA ramp so
                        # the clock un-throttles early.
                        emit_bases(item, pb=pb)
                    q1.append((item, pb))
                    step_epis(1)
                    if len(q1) > 1:
                        it, pb0 = q1.pop(0)
                        x_t, bsb = emit_mult((it, pb0))
                        q2.append((it, x_t, bsb))
                    if len(q2) > 1:
                        it, x_t, bsb = q2.pop(0)
                        x_t = emit_mult2((it, x_t, bsb))
                        q3.append((it, x_t))
                    step_epis(1)
                    if len(q3) > 2:
                        emit_scatter(q3.pop(0))
                    if pidx > 10:
                        step_epis(1)
                # drain
                while q1:
                    it, pb0 = q1.pop(0)
                    x_t, bsb = emit_mult((it, pb0))
                    q2.append((it, x_t, bsb))
                while q2:
                    it, x_t, bsb = q2.pop(0)
                    x_t = emit_mult2((it, x_t, bsb))
                    q3.append((it, x_t))
                while q3:
                    emit_scatter(q3.pop(0))
                    step_epis(2)
                if not last_q:
                    # drain any leftover of the previous quad's epilogue
                    # before queueing this quad's (strict sequencing).
                    while epi_q:
                        step_epis(1)
                    epi_q.append(epilogue_gen(q, zt_sb))
            while epi_q:
                step_epis(1)

    nc.compile()
    return nc


def _get_nc(t_atom, K):
    key = (t_atom, K)
    if key not in _NC_CACHE:
        _NC_CACHE[key] = _build_nc(t_atom, K)
    return _NC_CACHE[key]


# ----------------------------------------------------------------------------
# Entry point
# ----------------------------------------------------------------------------

def kernel(h, m, basis_rad, idx_atom, W_rbf, W_in, res_W1, res_W2):
    from concourse.bass_utils import run_bass_kernel_spmd

    m = np.asarray(m, dtype=np.float32)
    basis_rad = np.asarray(basis_rad, dtype=np.float32)
    idx = np.asarray(idx_atom).astype(np.int64)
    W_rbf = np.asarray(W_rbf, dtype=np.float32)
    W_in = np.asarray(W_in, dtype=np.float32)
    res_W1 = np.asarray(res_W1, dtype=np.float32)
    res_W2 = np.asarray(res_W2, dtype=np.float32)
    n_atoms = np.asarray(h).shape[0]

    layout = _pack_layout(idx, n_atoms, N_CORES, T_ATOM)
    in_maps = _build_in_maps(
        m, basis_rad, layout, W_rbf, W_in, res_W1, res_W2, N_CORES, T_ATOM
    )
    nc = _get_nc(T_ATOM, layout["K"])

    trace = os.environ.get("KERNEL_TRACE", "0") == "1"
    res = run_bass_kernel_spmd(
        nc, in_maps, core_ids=list(range(N_CORES)), trace=trace
    )
    if trace and res.exec_time_ns is not None:
        print(f"HW exec time: {res.exec_time_ns} ns", file=sys.stderr)
        kernel.last_exec_time_ns = res.exec_time_ns
    kernel.last_results = res
    return _unpack_output(res.results, layout, n_atoms, N_CORES, T_ATOM)


# revision 46
# speedup vs baseline: 1.0300x; 1.0300x over previous
"""Trainium2 Bass kernel for GemNet AtomUpdateBlock (gnn_message_passing).

Computation (per reference):
    bases = basis_rad @ W_rbf              # [E, De]
    x     = m * bases                      # [E, De]
    z     = segment_sum(x, idx_atom, A)    # [A, De]
    x     = silu(z @ W_in)                 # [A, Da]
    3x residual: x = (x + silu(silu(x W1) W2)) / sqrt(2)

Distribution strategy: shard EDGES BY DESTINATION ATOM. The host bins the
atoms into 8 cores x T_ATOM tiles of <=128 atoms (balanced by edge count),
sorts/pads each tile's edges into K 128-edge groups, and each core computes
the segment-sum + atom MLP for its own atoms only. No collective needed;
outputs are disjoint atom slices.

v2 (vs v1 baseline at ~292-347us):
 - bases matmuls (K=16) are 2-way ROW-TILED: two consecutive 128-edge
   groups' basis blocks sit at partitions 0-15 / 32-47 and run as
   concurrent 32-row PE tiles into two PSUM banks of one [P,1024] tile.
 - the x = m*bases multiply is split across engines per pack:
   route A: ACT copies the [128,1024] f32 PSUM pair to SBUF bf16, then
   DVE (or GPSIMD for one pack per subtile) does a bf16 2x-rate TT.
   route B: DVE does the TT directly from PSUM at 1x rate.
   A 3-stage software pipeline (bases -> evac/mult -> scatter) keeps the
   PE dense so the HAM clock gate stays at 8/8.
 - z evac: PSUM->SBUF bf16 copy, 4 bf16 PE transposes into one PSUM bank,
   one dense DVE copy out (zt layout is subtile-major; the MLP matmuls
   read it with a strided rhs AP).
 - epilogue u/v/w PSUM tiles are [P,1024] so each silu is one batched
   [128,1024] ACT instruction; residual skip-adds and the final 1/sqrt(2)^3
   scale run on the otherwise-idle GPSIMD engine.
"""

import math
import os
import sys

import numpy as np
import ml_dtypes

BF16 = ml_dtypes.bfloat16

P = 128
N_CORES = 8
DE, DA, DR, NH = 512, 256, 16, 3
T_ATOM = 20  # atom tiles per core (each up to 128 atoms); divisible by 4
INV_SQRT_2 = 0.7071067811865476

_NC_CACHE = {}
SILU_NATIVE = True  # False: sigmoid+mult (for CoreSim, which lacks Silu)


# ----------------------------------------------------------------------------
# Host-side packing
# ----------------------------------------------------------------------------

def _pack_layout(idx, n_atoms, n_cores, t_atom):
    E = idx.shape[0]
    n_bins = n_cores * t_atom
    counts = np.bincount(idx, minlength=n_atoms)

    order = np.argsort(-counts, kind="stable")
    n_rounds = math.ceil(n_atoms / n_bins)
    pad = n_rounds * n_bins - n_atoms
    padded = np.concatenate([order, np.full(pad, -1, dtype=order.dtype)])
    grid = padded.reshape(n_rounds, n_bins)
    grid[1::2] = grid[1::2, ::-1]  # snake-deal: balances edges and atoms
    bin_of_atom = np.empty(n_atoms, dtype=np.int64)
    slot_of_atom = np.empty(n_atoms, dtype=np.int64)
    valid = grid >= 0
    bin_idx = np.broadcast_to(np.arange(n_bins), grid.shape)
    round_idx = np.broadcast_to(np.arange(n_rounds)[:, None], grid.shape)
    bin_of_atom[grid[valid]] = bin_idx[valid]
    slot_of_atom[grid[valid]] = round_idx[valid]
    assert np.bincount(bin_of_atom, minlength=n_bins).max() <= P

    ebin = bin_of_atom[idx]
    eslot = slot_of_atom[idx]
    eorder = np.argsort(ebin * (P + 1) + eslot, kind="stable")
    ebin_sorted = ebin[eorder]
    bin_counts = np.bincount(ebin_sorted, minlength=n_bins)
    K = max(1, math.ceil(bin_counts.max() / P))
    bin_starts = np.zeros(n_bins + 1, dtype=np.int64)
    np.cumsum(bin_counts, out=bin_starts[1:])
    pos_in_bin = np.arange(E) - bin_starts[ebin_sorted]

    core_of_bin = np.arange(n_bins) // t_atom
    tile_of_bin = np.arange(n_bins) % t_atom
    return dict(
        K=K,
        eorder=eorder,
        core_of_edge=core_of_bin[ebin_sorted],
        flat_slot=tile_of_bin[ebin_sorted] * (K * P) + pos_in_bin,
        rel_of_edge=eslot[eorder].astype(np.int64),
        bin_of_atom=bin_of_atom,
        slot_of_atom=slot_of_atom,
        core_of_bin=core_of_bin,
        tile_of_bin=tile_of_bin,
    )


def _pack_weights(W_rbf, W_in, res_W1, res_W2):
    Ci, Cj = DE // P, DA // P
    Cr = DA // P
    win = W_in.reshape(Ci, P, Cj, P).transpose(1, 0, 2, 3).reshape(P, Ci * Cj * P)
    blocks = []
    c = INV_SQRT_2
    for l in range(NH):
        w1 = (res_W1[l] * (c ** l)).astype(np.float32)
        w2 = res_W2[l].astype(np.float32)
        for W in (w1, w2):
            blocks.append(
                W.reshape(Cr, P, Cr, P).transpose(1, 0, 2, 3).reshape(P, Cr * Cr * P)
            )
    wres = np.concatenate(blocks, axis=1)
    # W_rbf replicated at partitions 0-15 and 32-47 for 2-way row tiling
    wrbf2 = np.zeros((64, DE), dtype=BF16)
    wrbf2[0:DR] = W_rbf.astype(BF16)
    wrbf2[32:32 + DR] = W_rbf.astype(BF16)
    return (
        np.ascontiguousarray(wrbf2),
        np.ascontiguousarray(win, dtype=BF16),
        np.ascontiguousarray(wres, dtype=BF16),
    )


def _build_in_maps(m, basis_rad, layout, W_rbf, W_in, res_W1, res_W2, n_cores, t_atom):
    K = layout["K"]
    cap = t_atom * K * P
    ncols = t_atom * K
    npairs = K // 2
    npacks = npairs + (K % 2)
    eorder = layout["eorder"]
    core_of_edge = layout["core_of_edge"]
    flat_slot = layout["flat_slot"]
    rel = layout["rel_of_edge"]

    wrbf2, win, wres = _pack_weights(W_rbf, W_in, res_W1, res_W2)
    m_src = m[eorder]
    bas_src = basis_rad[eorder]
    ident = np.eye(P, dtype=BF16)

    in_maps = []
    for c in range(n_cores):
        sel = core_of_edge == c
        fs = flat_slot[sel]
        m_pack = np.zeros((cap, DE), dtype=BF16)
        m_pack[fs] = m_src[sel].astype(BF16)
        # partition-major: m2[p, col*DE + d] = m_pack[col*P + p, d] so each
        # partition's per-atom-tile DMA read is fully contiguous
        m_pack = np.ascontiguousarray(
            m_pack.reshape(ncols, P, DE).transpose(1, 0, 2).reshape(P, ncols * DE)
        )
        basT = np.zeros((DR, cap), dtype=BF16)
        basT[:, fs] = bas_src[sel].T.astype(BF16)
        # 2-way row-tiled layout: pack j of tile t holds group 2j at
        # partitions 0-15 and group 2j+1 (if any) at partitions 32-47.
        bas_pack = np.zeros((64, t_atom * npacks * P), dtype=BF16)
        for t in range(t_atom):
            for j in range(npacks):
                col0 = (t * npacks + j) * P
                k = 2 * j
                bas_pack[0:DR, col0:col0 + P] = basT[
                    :, (t * K + k) * P:(t * K + k + 1) * P
                ]
                if k + 1 < K:
                    bas_pack[32:32 + DR, col0:col0 + P] = basT[
                        :, (t * K + k + 1) * P:(t * K + k + 2) * P
                    ]
        rel_flat = np.full(cap, -1, dtype=np.int64)
        rel_flat[fs] = rel[sel]
        rel2 = rel_flat.reshape(ncols, P).T  # [p, col]
        s_host = (rel2[:, :, None] == np.arange(P)[None, None, :]).astype(BF16)
        in_maps.append(
            dict(
                m_pack=m_pack,
                bas_pack=np.ascontiguousarray(bas_pack),
                s_hot=np.ascontiguousarray(s_host.reshape(P, ncols * P)),
                wrbf2=wrbf2,
                win=win,
                wres=wres,
                ident=ident,
            )
        )
    return in_maps


def _unpack_output(results, layout, n_atoms, n_cores, t_atom):
    Cj = DA // P
    out = np.zeros((n_atoms, DA), dtype=np.float32)
    core_of_atom = layout["core_of_bin"][layout["bin_of_atom"]]
    row_of_atom = (
        layout["tile_of_bin"][layout["bin_of_atom"]] * P + layout["slot_of_atom"]
    )
    for c in range(n_cores):
        x = results[c]["out"].reshape(P, Cj, t_atom, P)
        x_core = x.transpose(2, 3, 1, 0).reshape(t_atom * P, DA)
        mask = core_of_atom == c
        out[mask] = x_core[row_of_atom[mask]]
    return out


# ----------------------------------------------------------------------------
# Bass kernel builder
# ----------------------------------------------------------------------------

def _build_nc(t_atom, K):
    import concourse.mybir as mybir
    import concourse.tile as tile
    from concourse import bacc

    f32 = mybir.dt.float32
    bf16 = mybir.dt.bfloat16
    Ci, Cj = DE // P, DA // P
    Cr = DA // P
    npairs = K // 2
    npacks = npairs + (K % 2)
    C3 = INV_SQRT_2 ** NH
    GAMMA = [float((1.0 / INV_SQRT_2) ** l) for l in range(NH)]
    assert t_atom % 4 == 0
    n_quads = t_atom // 4
    W4 = 4 * P  # atoms per epilogue quad

    nc = bacc.Bacc(
        "TRN2",
        target_bir_lowering=False,
        debug=False,
        enable_asserts=False,
        num_devices=N_CORES,
    )
    d_m = nc.dram_tensor("m_pack", [P, t_atom * K * DE], bf16, kind="ExternalInput")
    d_bas = nc.dram_tensor(
        "bas_pack", [64, t_atom * npacks * P], bf16, kind="ExternalInput"
    )
    d_s = nc.dram_tensor("s_hot", [P, t_atom * K * P], bf16, kind="ExternalInput")
    d_wrbf2 = nc.dram_tensor("wrbf2", [64, DE], bf16, kind="ExternalInput")
    d_win = nc.dram_tensor("win", [P, Ci * Cj * P], bf16, kind="ExternalInput")
    d_wres = nc.dram_tensor(
        "wres", [P, NH * 2 * Cr * Cr * P], bf16, kind="ExternalInput"
    )
    d_ident = nc.dram_tensor("ident", [P, P], bf16, kind="ExternalInput")
    d_out = nc.dram_tensor("out", [P, Cj * t_atom * P], f32, kind="ExternalOutput")

    with tile.TileContext(nc) as tc:
        with (
            tc.tile_pool(name="const", bufs=1) as const_p,
            tc.tile_pool(name="bas", bufs=5) as bas_p,
            tc.tile_pool(name="m", bufs=5) as m_p,
            tc.tile_pool(name="x", bufs=4) as x_p,
            tc.tile_pool(name="bsb", bufs=3) as bsb_p,
            tc.tile_pool(name="s", bufs=5) as s_p,
            tc.tile_pool(name="zsb", bufs=3) as zsb_p,
            tc.tile_pool(name="ztsb", bufs=2) as ztsb_p,
            tc.tile_pool(name="act", bufs=4) as act_p,
            tc.tile_pool(name="outp", bufs=3) as out_p,
            tc.tile_pool(name="ps_z", bufs=2, space="PSUM") as psz_p,
            tc.tile_pool(name="ps_bases", bufs=2, space="PSUM") as psb_p,
            tc.tile_pool(name="ps_misc", bufs=1, space="PSUM") as psm_p,
        ):
            # Resident constants
            wrbf_sb = const_p.tile([64, DE], bf16, tag="wrbf2")
            nc.sync.dma_start(out=wrbf_sb[:], in_=d_wrbf2[:])
            win_sb = const_p.tile([P, Ci * Cj * P], bf16, tag="win")
            nc.sync.dma_start(out=win_sb[:], in_=d_win[:])
            wres_sb = const_p.tile([P, NH * 2 * Cr * Cr * P], bf16, tag="wres")
            nc.sync.dma_start(out=wres_sb[:], in_=d_wres[:])
            ident = const_p.tile([P, P], bf16, tag="ident")
            nc.sync.dma_start(out=ident[:], in_=d_ident[:])

            # HAM warmup: dense back-to-back matmuls on resident weights
            # upclock the PE (4/8 -> 8/8) while the first DMAs stream.
            warm_ps = psm_p.tile(
                [P, Cj * W4], f32, space="PSUM", tag="misc", name="warm"
            )
            for w in range(44):
                nc.tensor.matmul(
                    out=warm_ps[:, 0:W4],
                    lhsT=win_sb[:, (w % 8) * P : (w % 8 + 1) * P],
                    rhs=win_sb[:, 0:W4],
                    start=True,
                    stop=True,
                )

            _ctr = [0]

            def emit_silu(out_ap, in_ps_ap):
                if SILU_NATIVE:
                    nc.scalar.activation(
                        out=out_ap, in_=in_ps_ap,
                        func=mybir.ActivationFunctionType.Silu,
                    )
                else:
                    _ctr[0] += 1
                    sg = act_p.tile(
                        [P, in_ps_ap.free_size()], f32, tag="sig",
                        name=f"sig{_ctr[0]}"
                    )
                    nc.scalar.activation(
                        out=sg[:], in_=in_ps_ap,
                        func=mybir.ActivationFunctionType.Sigmoid,
                    )
                    nc.vector.tensor_tensor(
                        out=out_ap, in0=in_ps_ap, in1=sg[:],
                        op=mybir.AluOpType.mult,
                    )

            def epilogue_gen(q, zt_sb, s0=0, ns=4, delay=5):
                """Epilogue over atom columns of subtiles [s0, s0+ns) of quad
                q, emitted as units interleavable with the edge stream. The
                epilogue is column-separable by subtile, so the last quad
                runs it as per-subtile chains that start right after each
                subtile's evac instead of after the whole quad."""
                ztr = zt_sb.rearrange("p (s c x) -> p s c x", s=4, c=Ci)
                W = ns * P  # atom columns processed by this chain
                # Delay the first u matmul a few packs so the source zt
                # (previous quad's last-subtile evac) has completed and the
                # in-order PE queue never blocks on it.
                for _ in range(delay):
                    yield
                u_ps = psm_p.tile(
                    [P, Cj * W], f32, space="PSUM", tag="misc",
                    name=f"ups{q}_{s0}"
                )
                for j in range(Cj):
                    for c in range(Ci):
                        fi = c * Cj + j
                        nc.tensor.matmul(
                            out=u_ps[:, j * W:(j + 1) * W],
                            lhsT=win_sb[:, fi * P : (fi + 1) * P],
                            rhs=ztr[:, s0 : s0 + ns, c, :],
                            start=(c == 0),
                            stop=(c == Ci - 1),
                        )
                        yield
                X = act_p.tile([P, Cj * W], bf16, tag="X", name=f"X{q}_{s0}_0")
                emit_silu(X[:], u_ps[:])
                yield
                yield
                yield
                for l in range(NH):
                    v_ps = psm_p.tile(
                        [P, Cr * W], f32, space="PSUM", tag="misc",
                        name=f"vps{q}_{s0}_{l}"
                    )
                    for j in range(Cr):
                        for i in range(Cr):
                            fi = ((l * 2 + 0) * Cr + i) * Cr + j
                            nc.tensor.matmul(
                                out=v_ps[:, j * W:(j + 1) * W],
                                lhsT=wres_sb[:, fi * P : (fi + 1) * P],
                                rhs=X[:, i * W : (i + 1) * W],
                                start=(i == 0),
                                stop=(i == Cr - 1),
                            )
                            yield
                    u1 = act_p.tile(
                        [P, Cr * W], bf16, tag="X", name=f"u1_{q}_{s0}_{l}"
                    )
                    emit_silu(u1[:], v_ps[:])
                    yield
                    yield
                    w_ps = psm_p.tile(
                        [P, Cr * W], f32, space="PSUM", tag="misc",
                        name=f"wps{q}_{s0}_{l}"
                    )
                    for j in range(Cr):
                        for i in range(Cr):
                            fi = ((l * 2 + 1) * Cr + i) * Cr + j
                            nc.tensor.matmul(
                                out=w_ps[:, j * W:(j + 1) * W],
                                lhsT=wres_sb[:, fi * P : (fi + 1) * P],
                                rhs=u1[:, i * W : (i + 1) * W],
                                start=(i == 0),
                                stop=(i == Cr - 1),
                            )
                            yield
                    Y = act_p.tile(
                        [P, Cr * W], bf16, tag="X", name=f"Y{q}_{s0}_{l}"
                    )
                    emit_silu(Y[:], w_ps[:])
                    yield
                    yield
                    Xn = act_p.tile(
                        [P, Cr * W], bf16, tag="X", name=f"X{q}_{s0}_{l + 1}"
                    )
                    nc.vector.scalar_tensor_tensor(
                        out=Xn[:],
                        in0=Y[:],
                        scalar=GAMMA[l],
                        in1=X[:],
                        op0=mybir.AluOpType.mult,
                        op1=mybir.AluOpType.add,
                    )
                    X = Xn
                    yield
                o_t = out_p.tile([P, Cj * W], f32, tag="out", name=f"ot{q}_{s0}")
                nc.scalar.mul(out=o_t[:], in_=X[:], mul=float(C3))
                for j in range(Cj):
                    nc.sync.dma_start(
                        out=d_out[
                            :,
                            (j * t_atom + 4 * q + s0) * P
                            : (j * t_atom + 4 * q + s0 + ns) * P,
                        ],
                        in_=o_t[:, j * W : (j + 1) * W],
                    )
                yield

            # Pending epilogue generators, stepped strictly SEQUENTIALLY
            # (one exhausts before the next starts): concurrent generators
            # sharing the single-buffer psm pool deadlock the in-order PE
            # queue (gen B's tile alloc waits on gen A's not-yet-emitted
            # reader).
            epi_q = []

            def step_epis(n=1):
                for _ in range(n):
                    while epi_q:
                        if next(epi_q[0], StopIteration) is StopIteration:
                            epi_q.pop(0)
                            continue
                        break

            # Flat subtile stream: DMAs for subtile ts are emitted two
            # subtiles ahead of use so the sync queue never bunches a whole
            # quad's transfers at a quad boundary (that bunching caused
            # ~12us PE stalls at every quad edge).
            sub_state = {}

            def emit_subtile_dmas(ts):
                if ts >= 4 * n_quads or ts in sub_state:
                    return
                bas_sb = bas_p.tile(
                    [64, npacks * P], bf16, tag="bas", name=f"bas{ts}"
                )
                nc.sync.dma_start(
                    out=bas_sb[:],
                    in_=d_bas[:, ts * npacks * P : (ts + 1) * npacks * P],
                )
                m_t = m_p.tile([P, K * DE], bf16, tag="m", name=f"mt{ts}")
                nc.sync.dma_start(
                    out=m_t[:], in_=d_m[:, ts * K * DE : (ts + 1) * K * DE]
                )
                s_t = s_p.tile([P, K * P], bf16, tag="s", name=f"st{ts}")
                nc.sync.dma_start(
                    out=s_t[:], in_=d_s[:, ts * K * P : (ts + 1) * K * P]
                )
                z_ps = psz_p.tile(
                    [P, DE], f32, space="PSUM", tag="z", name=f"zps{ts}"
                )
                sub_state[ts] = (bas_sb, m_t, s_t, z_ps)

            for ts in range(3):
                emit_subtile_dmas(ts)

            for q in range(n_quads):
                subs = {}

                def do_evac(sub):
                    """z psum -> sbuf bf16 -> 4 bf16 PE transposes -> one
                    dense DVE copy into the quad's zt_sb (subtile-major)."""
                    t = 4 * q + sub
                    z_sb = zsb_p.tile([P, DE], bf16, tag="zsb", name=f"zsb{t}")
                    nc.scalar.copy(out=z_sb[:], in_=subs[sub][3][:])
                    zt_ps = psm_p.tile(
                        [P, DE], bf16, space="PSUM", tag="misc", name=f"ztp{t}"
                    )
                    for c in range(Ci):
                        nc.tensor.transpose(
                            out=zt_ps[:, c * P : (c + 1) * P],
                            in_=z_sb[:, c * P : (c + 1) * P],
                            identity=ident[:],
                        )
                    nc.vector.tensor_copy(
                        out=zt_sb[:, sub * DE : (sub + 1) * DE], in_=zt_ps[:]
                    )

                zt_sb = ztsb_p.tile([P, 4 * Ci * P], bf16, tag="ztsb")

                # pack list with routes:
                #  'A'  : ACT copies PSUM pair -> SBUF bf16, DVE does 2x TT
                #  'Agp': same but GPSIMD does the TT (one pack per subtile)
                #  'B'  : DVE tensor_tensor straight from PSUM (1x)
                packs = []
                for sub in range(4):
                    t = 4 * q + sub
                    for j in range(npacks):
                        if j >= npairs:
                            route = "B"  # odd-K single group
                        elif j == 0:
                            route = "Agp"
                        elif j % 2 == 0:
                            route = "A"
                        elif j == npairs - 1 and t % 2 == 0:
                            route = "A"
                        else:
                            route = "B"
                        packs.append((sub, j, route))

                def emit_bases(item, pb=None):
                    sub, j, route = item
                    bas_sb = subs[sub][0]
                    if pb is None:
                        pb = psb_p.tile(
                            [P, 2 * DE], f32, space="PSUM", tag="bases",
                            name=f"pb{q}_{sub}_{j}"
                        )
                    nc.tensor.matmul(
                        out=pb[:, 0:DE],
                        lhsT=bas_sb[0:DR, j * P : (j + 1) * P],
                        rhs=wrbf_sb[0:DR, :],
                        start=True,
                        stop=True,
                    )
                    if j < npairs:
                        nc.tensor.matmul(
                            out=pb[:, DE : 2 * DE],
                            lhsT=bas_sb[32 : 32 + DR, j * P : (j + 1) * P],
                            rhs=wrbf_sb[32 : 32 + DR, :],
                            start=True,
                            stop=True,
                        )
                    return pb

                def emit_mult(st):
                    """Stage 1: route A -> ACT copy; route B -> DVE TT."""
                    (sub, j, route), pb = st
                    ngr = 2 if j < npairs else 1
                    w = ngr * DE
                    m_t = subs[sub][1]
                    k0 = 2 * j
                    if route == "B":
                        x_t = x_p.tile(
                            [P, 2 * DE], bf16, tag="x", name=f"x{q}_{sub}_{j}"
                        )
                        nc.vector.tensor_tensor(
                            out=x_t[:, 0:w],
                            in0=pb[:, 0:w],
                            in1=m_t[:, k0 * DE : k0 * DE + w],
                            op=mybir.AluOpType.mult,
                        )
                        return x_t, None
                    bsb = bsb_p.tile(
                        [P, 2 * DE], bf16, tag="bsb", name=f"bsb{q}_{sub}_{j}"
                    )
                    nc.scalar.copy(out=bsb[:, 0:w], in_=pb[:, 0:w])
                    return None, bsb

                def emit_mult2(st2):
                    """Stage 2: route A's bf16 TT on DVE or GPSIMD."""
                    (sub, j, route), x_t, bsb = st2
                    if route == "B":
                        return x_t
                    ngr = 2 if j < npairs else 1
                    w = ngr * DE
                    m_t = subs[sub][1]
                    k0 = 2 * j
                    x_t = x_p.tile(
                        [P, 2 * DE], bf16, tag="x", name=f"x{q}_{sub}_{j}"
                    )
                    eng = nc.gpsimd if route == "Agp" else nc.vector
                    eng.tensor_tensor(
                        out=x_t[:, 0:w],
                        in0=bsb[:, 0:w],
                        in1=m_t[:, k0 * DE : k0 * DE + w],
                        op=mybir.AluOpType.mult,
                    )
                    return x_t

                last_q = q == n_quads - 1

                def emit_scatter(st3):
                    (sub, j, route), x_t = st3
                    s_t, z_ps = subs[sub][2], subs[sub][3]
                    ngr = 2 if j < npairs else 1
                    k0 = 2 * j
                    for g in range(ngr):
                        k = k0 + g
                        nc.tensor.matmul(
                            out=z_ps[:],
                            lhsT=s_t[:, k * P : (k + 1) * P],
                            rhs=x_t[:, g * DE : (g + 1) * DE],
                            start=(k == 0),
                            stop=(k == K - 1),
                        )
                    if j == npacks - 1:
                        do_evac(sub)

                q1, q2, q3 = [], [], []
                for pidx, item in enumerate(packs):
                    if item[1] == 0:
                        # entering a new subtile: its tiles were prefetched;
                        # kick off the DMAs for the subtile 3 ahead.
                        subs[item[0]] = sub_state[4 * q + item[0]]
                        emit_subtile_dmas(4 * q + item[0] + 3)
                    pb = emit_bases(item)
                    if q == 0 and 1 <= pidx <= 12:
                        # HAM warm-boost: duplicate the bases matmuls
                        # (idempotent PSUM overwrites) to keep the PE's
                        # activity window saturated during the DMA ramp so
                        # the clock un-throttles early.
                        emit_bases(item, pb=pb)
                    q1.append((item, pb))
                    step_epis(1)
                    if len(q1) > 1:
                        it, pb0 = q1.pop(0)
                        x_t, bsb = emit_mult((it, pb0))
                        q2.append((it, x_t, bsb))
                    if len(q2) > 1:
                        it, x_t, bsb = q2.pop(0)
                        x_t = emit_mult2((it, x_t, bsb))
                        q3.append((it, x_t))
                    step_epis(1)
                    if len(q3) > 2:
                        emit_scatter(q3.pop(0))
                    if pidx > 16:
                        step_epis(1)
                # drain
                while q1:
                    it, pb0 = q1.pop(0)
                    x_t, bsb = emit_mult((it, pb0))
                    q2.append((it, x_t, bsb))
                while q2:
                    it, x_t, bsb = q2.pop(0)
                    x_t = emit_mult2((it, x_t, bsb))
                    q3.append((it, x_t))
                while q3:
                    emit_scatter(q3.pop(0))
                    step_epis(2)
                # drain any leftover of the previous quad's epilogue before
                # queueing this quad's (strict sequencing on the psm pool).
                while epi_q:
                    step_epis(1)
                epi_q.append(epilogue_gen(q, zt_sb))
            while epi_q:
                step_epis(1)

    nc.compile()
    return nc


def _get_nc(t_atom, K):
    key = (t_atom, K)
    if key not in _NC_CACHE:
        _NC_CACHE[key] = _build_nc(t_atom, K)
    return _NC_CACHE[key]


# ----------------------------------------------------------------------------
# Entry point
# ----------------------------------------------------------------------------

def kernel(h, m, basis_rad, idx_atom, W_rbf, W_in, res_W1, res_W2):
    from concourse.bass_utils import run_bass_kernel_spmd

    m = np.asarray(m, dtype=np.float32)
    basis_rad = np.asarray(basis_rad, dtype=np.float32)
    idx = np.asarray(idx_atom).astype(np.int64)
    W_rbf = np.asarray(W_rbf, dtype=np.float32)
    W_in = np.asarray(W_in, dtype=np.float32)
    res_W1 = np.asarray(res_W1, dtype=np.float32)
    res_W2 = np.asarray(res_W2, dtype=np.float32)
    n_atoms = np.asarray(h).shape[0]

    layout = _pack_layout(idx, n_atoms, N_CORES, T_ATOM)
    in_maps = _build_in_maps(
        m, basis_rad, layout, W_rbf, W_in, res_W1, res_W2, N_CORES, T_ATOM
    )
    nc = _get_nc(T_ATOM, layout["K"])

    trace = os.environ.get("KERNEL_TRACE", "0") == "1"
    res = run_bass_kernel_spmd(
        nc, in_maps, core_ids=list(range(N_CORES)), trace=trace
    )
    if trace and res.exec_time_ns is not None:
        print(f"HW exec time: {res.exec_time_ns} ns", file=sys.stderr)
        kernel.last_exec_time_ns = res.exec_time_ns
    kernel.last_results = res
    return _unpack_output(res.results, layout, n_atoms, N_CORES, T_ATOM)
